# revision 28
# baseline (speedup 1.0000x reference)
"""Trainium2 Bass kernel for nn_EEGMI_RWKV_ResNet_Model.

Sharding: data-parallel over batch. B=32 -> 4 batches on each of 8 cores.
All parameters are baked into the NEFF via inline_tensor (loaded to HBM at
model load); only x ships per exec as (BL, 64, T) bf16.

Structure (per core):
  - band conv (depthwise 5-tap) on DVE with attention-prescaled coefficients
  - resnet convs on PE (contraction blocks), residual adds folded into PSUM
    via identity matmuls, relu+bias via ACT on the PSUM->SBUF move
  - rwkv: token-mix folded into row-scaled weight copies (W' = W diag(tm)),
    per-batch mix bias via tiny matmuls; wkv scan on DVE
    (tensor_tensor_scan); LN stats via ones-matmuls + Ln/Exp inverse-stddev;
    per-t broadcast via K=1 matmuls
  - emission is software-pipelined: batches 0,1 flow band->conv->rwkv while
    batches 2,3's convs are pumped into the PE stream between rwkv groups.
"""
import numpy as np
import ml_dtypes

import concourse.bass as bass
import concourse.bacc as bacc
import concourse.tile as tile
from concourse import mybir
from concourse.bass_utils import run_bass_kernel_spmd

EPS = 1e-5
B, T, C = 32, 2048, 64
NB, C5, H, L, NBLK, NCLS = 5, 320, 128, 3, 2, 4
CP = 384
NCORE = 8
BL = B // NCORE
NCH = 4
CH = 512
TP = T + 4      # padded width for band conv input
TF = T + 4      # feat tiles width (data cols 2..2050)

PERM = np.array([(o % 64) * 5 + (o // 64) for o in range(C5)], dtype=np.int64)

F32 = mybir.dt.float32
F32R = mybir.dt.float32r
BF16 = mybir.dt.bfloat16
AF = mybir.ActivationFunctionType
ALU = mybir.AluOpType
bf16np = ml_dtypes.bfloat16


# ---------------------------------------------------------------------------
# host-side weight preprocessing (numpy only)
# ---------------------------------------------------------------------------

def _prep_weights(inp):
    f32 = np.float32
    out = {}

    bw = np.asarray(inp['band_w'], f32)[PERM, 0, :]
    bb = np.asarray(inp['band_b'], f32)[PERM]
    bw_pad = np.zeros((CP, 5), f32); bw_pad[:C5] = bw
    bb_pad = np.zeros((CP,), f32); bb_pad[:C5] = bb
    band_coef = bw_pad.reshape(3, 128, 5)

    bw_raw = np.asarray(inp['band_w'], f32)[:, 0, :].reshape(C, NB, 5)
    denom = f32(1.0 / (NB * T))
    A = bw_raw.sum(axis=(1, 2)) * denom
    E0 = -(bw_raw[:, :, 3] + bw_raw[:, :, 4]).sum(1) * denom
    E1 = -(bw_raw[:, :, 4]).sum(1) * denom
    E2 = -(bw_raw[:, :, 0]).sum(1) * denom
    E3 = -(bw_raw[:, :, 0] + bw_raw[:, :, 1]).sum(1) * denom
    Bb = np.asarray(inp['band_b'], f32).reshape(C, NB).mean(1)

    attn_rhs = np.zeros((65, 64), f32)
    attn_rhs[:64] = np.asarray(inp['attn_w'], f32).T
    attn_rhs[64] = np.asarray(inp['attn_b'], f32)
    out['attn_rhs'] = attn_rhs

    # q2-packed conv weights: per c4, 24 lhsT blocks of (128,128):
    #   idx (k*2+q)*3+mm  for q in {0,1}, k in 0..2  (18 blocks)
    #   idx 18 + mm*2 + j for the q=2 block: j=0 packs [k0-rows; k2-rows]
    #   (rhs rows 64:128 hold a shift-by-2 duplicate of block-2 channels),
    #   j=1 is [k1-rows; zeros].
    res_lhsT = np.zeros((4, 24, 128, 128), f32)
    res_bias = np.zeros((4, CP), f32)
    ci = 0
    for blk in range(NBLK):
        for lyr in range(2):
            W = np.asarray(inp['res_w'], np.float32)[blk, lyr]
            g = np.asarray(inp['res_bn_g'], f32)[blk, lyr]
            b = np.asarray(inp['res_bn_b'], f32)[blk, lyr]
            m = np.asarray(inp['res_bn_m'], f32)[blk, lyr]
            v = np.asarray(inp['res_bn_v'], f32)[blk, lyr]
            inv = g / np.sqrt(v + EPS)
            Wf = W * inv[:, None, None]
            bf = b - m * inv
            Wp = Wf[PERM][:, PERM]
            Wpad = np.zeros((CP, CP, 3), f32); Wpad[:C5, :C5] = Wp
            bpad = np.zeros((CP,), f32); bpad[:C5] = bf[PERM]
            res_bias[ci] = bpad
            WT = Wpad.transpose(1, 0, 2)   # (in_ch, out_ch, k)
            for k in range(3):
                for q in range(2):
                    for mm in range(3):
                        res_lhsT[ci, (k*2+q)*3+mm] = \
                            WT[q*128:(q+1)*128, mm*128:(mm+1)*128, k]
            for mm in range(3):
                mc = slice(mm*128, (mm+1)*128)
                res_lhsT[ci, 18 + mm*2 + 0, 0:64] = WT[256:320, mc, 0]
                res_lhsT[ci, 18 + mm*2 + 0, 64:128] = WT[256:320, mc, 2]
                res_lhsT[ci, 18 + mm*2 + 1, 0:64] = WT[256:320, mc, 1]
            ci += 1
    out['res_lhsT'] = np.ascontiguousarray(
        res_lhsT.transpose(2, 0, 1, 3)).astype(bf16np)

    pw = np.asarray(inp['proj_w'], f32)[:, PERM]
    pw_pad = np.zeros((H, CP), f32); pw_pad[:, :C5] = pw
    out['proj_lhsT'] = np.ascontiguousarray(
        pw_pad.T.reshape(3, 128, H).transpose(1, 0, 2)).astype(bf16np)

    # token-mix fold: W'[w] = W diag(tm_w)  (lhsT rows scaled by tm);
    # bias weights W''[w] = W diag((1-tm_w)/T) for the per-batch mean term.
    rwkv_lhsT = np.zeros((L, 4, H, H), f32)
    rwkv_blhsT = np.zeros((L, 3, H, H), f32)
    tms = [np.asarray(inp['tmk'], f32), np.asarray(inp['tmv'], f32),
           np.asarray(inp['tmr'], f32)]
    ws = [np.asarray(inp['wk'], f32), np.asarray(inp['wv'], f32),
          np.asarray(inp['wr'], f32)]
    for l in range(L):
        for w in range(3):
            rwkv_lhsT[l, w] = ws[w][l].T * tms[w][l][:, None]
            rwkv_blhsT[l, w] = ws[w][l].T * ((1.0 - tms[w][l]) / T)[:, None]
        rwkv_lhsT[l, 3] = np.asarray(inp['wo'], f32)[l].T
    out['rwkv_lhsT'] = np.ascontiguousarray(
        rwkv_lhsT.transpose(2, 0, 1, 3)).astype(bf16np)
    # bias (mean-mix) weights stay f32: sums is a 2048-term reduction and
    # its contribution must not be quantized to bf16
    out['rwkv_blhsT'] = np.ascontiguousarray(
        rwkv_blhsT.transpose(2, 0, 1, 3))

    w1 = np.asarray(inp['cls_w1'], f32)
    out['cls1_lhsT'] = np.ascontiguousarray(w1.T.reshape(H, 2, 128))
    w2 = np.asarray(inp['cls_w2'], f32)
    out['cls2_lhsT'] = np.ascontiguousarray(
        w2.T.reshape(2, 128, NCLS).transpose(1, 0, 2))

    out['ident_bf'] = np.eye(128, dtype=f32).astype(bf16np)

    cols = {}
    def vec(name, v):
        cols[name] = np.asarray(v, f32)
    def pad128(v):
        o = np.zeros(128, f32); o[:len(v)] = v; return o

    for i in range(3):
        for k in range(5):
            vec(f'band_c{i}_{k}', band_coef[i, :, k])
    for i in range(3):
        vec(f'band_bias_{i}', bb_pad.reshape(3, 128)[i])
    vec('A', pad128(A)); vec('E0', pad128(E0)); vec('E1', pad128(E1))
    vec('E2', pad128(E2)); vec('E3', pad128(E3)); vec('Bb', pad128(Bb))
    for c4 in range(4):
        for mm in range(3):
            vec(f'res_b{c4}_{mm}', res_bias[c4, mm*128:(mm+1)*128])
    vec('proj_b', np.asarray(inp['proj_b'], f32))
    for l in range(L):
        vec(f'ln1g_{l}', np.asarray(inp['ln1g'], f32)[l])
        vec(f'ln1b_{l}', np.asarray(inp['ln1b'], f32)[l])
        vec(f'ln2g_{l}', np.asarray(inp['ln2g'], f32)[l])
        vec(f'ln2b_{l}', np.asarray(inp['ln2b'], f32)[l])
    vec('cls_b1a', np.asarray(inp['cls_b1'], f32)[:128])
    vec('cls_b1b', np.asarray(inp['cls_b1'], f32)[128:])
    vec('cls_b2', pad128(np.asarray(inp['cls_b2'], f32)))
    vec('eps', np.full(128, EPS, f32))

    names = list(cols.keys())
    out['cvec'] = np.ascontiguousarray(np.stack([cols[n] for n in names], 1))
    out['cvec_idx'] = {n: i for i, n in enumerate(names)}
    return out


# ---------------------------------------------------------------------------
# bass kernel builder
# ---------------------------------------------------------------------------

def _inline(nc, arr, name, dtype=None):
    h = nc.inline_tensor(np.ascontiguousarray(arr), name=name)
    if dtype is not None and dtype != h.dtype:
        mls = nc.lookup_mls(h)
        mls.dtype = dtype
        h = bass.DRamTensorHandle(h.name, list(arr.shape), dtype)
    return h


def _build_nc(nv, prep):
    nc = bacc.Bacc(None, target_bir_lowering=False)

    d_x = nc.dram_tensor('x', [BL, 64, T], BF16, kind='ExternalInput')
    d = {
        'cvec': _inline(nc, prep['cvec'], 'cvec'),
        'attn': _inline(nc, prep['attn_rhs'], 'attn_rhs', F32R),
        'res': _inline(nc, prep['res_lhsT'], 'res_lhsT'),
        'proj': _inline(nc, prep['proj_lhsT'], 'proj_lhsT'),
        'rwkv': _inline(nc, prep['rwkv_lhsT'], 'rwkv_lhsT'),
        'rwkvb': _inline(nc, prep['rwkv_blhsT'], 'rwkv_blhsT', F32R),
        'cls1': _inline(nc, prep['cls1_lhsT'], 'cls1_lhsT', F32R),
        'cls2': _inline(nc, prep['cls2_lhsT'], 'cls2_lhsT', F32R),
        'ident': _inline(nc, prep['ident_bf'], 'ident_bf'),
    }
    d_out = nc.dram_tensor('out', [NCLS, BL], F32, kind='ExternalOutput')

    with tile.TileContext(nc) as tc:
        _emit(nc, tc, d_x, d, d_out, nv, prep['cvec_idx'])
    nc.finalize()
    return nc


def _emit(nc, tc, d_x, d, d_out, nv, cvi):
    from contextlib import ExitStack

    ctx = ExitStack()
    with ctx:
        consts = ctx.enter_context(tc.tile_pool(name='consts', bufs=1))
        wres = ctx.enter_context(tc.tile_pool(name='wres', bufs=2))
        convp = ctx.enter_context(tc.tile_pool(name='convp', bufs=4))
        fo = ctx.enter_context(tc.tile_pool(name='fo', bufs=12))
        rw = ctx.enter_context(tc.tile_pool(name='rw', bufs=20))
        stats = ctx.enter_context(tc.tile_pool(name='stats', bufs=1))
        small = ctx.enter_context(tc.tile_pool(name='small', bufs=1))
        psum = ctx.enter_context(tc.tile_pool(name='psum', bufs=4, space='PSUM'))

        def fot(name):
            return fo.tile([128, TF], BF16, tag='fo', name=name)

        def rwt(name):
            return rw.tile([128, TF], BF16, tag='rw', name=name)

        # ---------------- constants -----------------
        cvec = consts.tile([128, nv], F32)
        nc.gpsimd.dma_start(out=cvec, in_=d['cvec'][:, :])

        def colap(name):
            i = cvi[name]
            return cvec[:, i:i+1]

        ones_l = consts.tile([128, 1], BF16)
        nc.vector.memset(ones_l, 1.0)
        decay = consts.tile([128, T], F32)
        nc.vector.memset(decay, 0.9)
        # f32r tiles cannot be memset directly; synthesize via ACT Copy
        ones_lf = consts.tile([128, 128], F32R)
        nc.scalar.activation(out=ones_lf, in_=decay[:, 0:128], func=AF.Copy,
                             bias=1.0, scale=0.0)

        attn_rhs = consts.tile([65, 64], F32R)
        nc.gpsimd.dma_start(out=attn_rhs, in_=d['attn'][:, :])
        w_proj = consts.tile([128, 3, H], BF16)
        nc.gpsimd.dma_start(out=w_proj, in_=d['proj'][...])
        w_rwkv = consts.tile([128, L, 4, H], BF16)
        nc.gpsimd.dma_start(out=w_rwkv, in_=d['rwkv'][...])
        w_rwkvb = consts.tile([128, L, 3, H], F32R)
        nc.gpsimd.dma_start(out=w_rwkvb, in_=d['rwkvb'][...])
        w_cls1 = consts.tile([128, 2, 128], F32R)
        nc.gpsimd.dma_start(out=w_cls1, in_=d['cls1'][...])
        w_cls2 = consts.tile([128, 2, NCLS], F32R)
        nc.gpsimd.dma_start(out=w_cls2, in_=d['cls2'][...])
        ident = consts.tile([128, 128], BF16)
        nc.gpsimd.dma_start(out=ident, in_=d['ident'][...])

        # shared LN stat tiles: pair 0 uses partitions 0:64, pair 1 64:128;
        # allocated once so row-disjoint ops never falsely serialize.
        sty = stats.tile([128, T], F32, tag='sty', name='sty')
        stq = stats.tile([128, T], F32R, tag='stq', name='stq')
        stv = stats.tile([128, T], F32R, tag='stv', name='stv')
        st3 = (sty, stq, stv)

        # streamed resnet conv weights: ring of 2, 8 loads (2 halves x 4)
        wres_tiles = {}

        def wres_load(tag):
            half, c4 = tag
            t = wres.tile([128, 24, 128], BF16, tag='wres',
                          name=f'wres{half}_{c4}')
            nc.sync.dma_start(out=t, in_=d['res'][:, c4, :, :])
            wres_tiles[tag] = t

        # ---------------- stage 1: load x ------------
        xdup = [convp.tile([128, TP], BF16, tag='xdup', name=f'xdup{b}')
                for b in range(BL)]
        S_b = small.tile([64, BL], F32)
        for b in range(BL):
            nc.gpsimd.memset(xdup[b][:, 0:2], 0.0)
            nc.gpsimd.memset(xdup[b][:, 2+T:4+T], 0.0)
            nc.sync.dma_start(out=xdup[b][0:64, 2:2+T], in_=d_x[b, :, :])
            nc.sync.dma_start(out=xdup[b][64:128, 2:2+T], in_=d_x[b, :, :])
            nc.vector.tensor_reduce(
                out=S_b[:, b:b+1], in_=xdup[b][0:64, 2:2+T],
                axis=mybir.AxisListType.X, op=ALU.add)
        wres_load((0, 0))
        wres_load((0, 1))

        # ---------------- attention ------------------------------------
        pooledT = small.tile([65, BL], F32R)
        nc.scalar.activation(out=pooledT[64:65, :], in_=S_b[0:1, 0:BL],
                             func=AF.Copy, bias=1.0, scale=0.0)
        for b in range(BL):
            p = pooledT[0:64, b:b+1]
            nc.vector.tensor_scalar(
                out=p, in0=S_b[:, b:b+1], scalar1=colap('A')[0:64],
                scalar2=colap('Bb')[0:64], op0=ALU.mult, op1=ALU.add)
            for name, cc in [('E0', 2), ('E1', 3), ('E2', T), ('E3', T+1)]:
                nc.vector.scalar_tensor_tensor(
                    out=p, in0=xdup[b][0:64, cc:cc+1],
                    scalar=colap(name)[0:64], in1=p,
                    op0=ALU.mult, op1=ALU.add)
        att_ps = psum.tile([64, BL], F32, tag='rwp', name='att_ps')
        nc.tensor.matmul(att_ps, attn_rhs, pooledT)
        attE = small.tile([64, BL], F32R)
        nc.scalar.activation(out=attE, in_=att_ps, func=AF.Exp)
        sum_ps = psum.tile([1, BL], F32, tag='rwp', name='sum_ps')
        nc.tensor.matmul(sum_ps, ones_lf[0:64, 0:1], attE)
        arec = small.tile([1, BL], F32R)
        with nc.allow_low_precision(reason='softmax denom in fp32r is fine'):
            nc.vector.reciprocal(out=arec, in_=sum_ps)
        bc_ps = psum.tile([64, BL], F32, tag='rwp', name='bc_ps')
        nc.tensor.matmul(bc_ps, ones_lf[0:1, 0:64], arec, tile_position=(0, 0))
        attT = small.tile([64, BL], F32)
        nc.vector.tensor_tensor(out=attT, in0=attE, in1=bc_ps, op=ALU.mult)
        # attention-scaled band coefficients + biases, per batch
        avec = [small.tile([128, 1], F32, tag='avec', bufs=4, name=f'avec{b}')
                for b in range(BL)]
        cfb = [small.tile([128, 15], F32, tag='cfb', bufs=4, name=f'cfb{b}')
               for b in range(BL)]
        bxa = [small.tile([128, 3], F32, tag='bxa', bufs=4, name=f'bxa{b}')
               for b in range(BL)]
        c0 = cvi['band_c0_0']
        bb0 = cvi['band_bias_0']
        for b in range(BL):
            nc.gpsimd.dma_start(out=avec[b][0:64, :], in_=attT[:, b:b+1])
            nc.gpsimd.dma_start(out=avec[b][64:128, :], in_=attT[:, b:b+1])
            nc.vector.tensor_scalar(
                out=cfb[b], in0=cvec[:, c0:c0+15], scalar1=avec[b],
                scalar2=None, op0=ALU.mult)
            nc.vector.tensor_scalar(
                out=bxa[b], in0=cvec[:, bb0:bb0+3], scalar1=avec[b],
                scalar2=None, op0=ALU.mult)

        # ---------------- band conv (DVE) -------------------------------
        F = [None] * BL

        def dup2(tile_):
            # rows 64:128 <- shift-by-2 duplicate of rows 0:64 (q2 packing)
            nc.sync.dma_start(out=tile_[64:128, 0:TF-2], in_=tile_[0:64, 2:TF])

        def band(b):
            F[b] = [fot(f'F{b}_{i}') for i in range(3)]
            for i in range(3):
                dst = F[b][i][:, 2:2+T]
                nc.gpsimd.memset(F[b][i][:, 0:2], 0.0)
                nc.gpsimd.memset(F[b][i][:, 2+T:4+T], 0.0)
                nc.vector.tensor_scalar(
                    out=dst, in0=xdup[b][:, 0:T],
                    scalar1=cfb[b][:, 5*i:5*i+1],
                    scalar2=bxa[b][:, i:i+1], op0=ALU.mult, op1=ALU.add)
                for k in range(1, 5):
                    nc.vector.scalar_tensor_tensor(
                        out=dst, in0=xdup[b][:, k:k+T],
                        scalar=cfb[b][:, 5*i+k:5*i+k+1], in1=dst,
                        op0=ALU.mult, op1=ALU.add)
            dup2(F[b][2])

        # ---------------- resnet conv stage ------------------------------
        O = [None] * BL

        def conv(c4, b, half):
            wt = wres_tiles[(half, c4)]
            if c4 == 0:
                O[b] = [fot(f'O{b}_{m}') for m in range(3)]
                for m in range(3):
                    nc.gpsimd.memset(O[b][m][:, 0:2], 0.0)
                    nc.gpsimd.memset(O[b][m][:, 2+T:4+T], 0.0)
            IN = F[b] if c4 in (0, 2) else O[b]
            OUT = O[b] if c4 in (0, 2) else F[b]
            residual = c4 in (1, 3)
            for m in range(3):
                pts = [psum.tile([128, CH], F32, tag='cvp',
                                 name=f'cv{c4}_{b}_{m}_{n}')
                       for n in range(NCH)]
                first = True
                for k in range(3):
                    for q in range(2):
                        lhsT = wt[:, (k*2+q)*3 + m, :]
                        for n, pt in enumerate(pts):
                            nc.tensor.matmul(
                                pt, lhsT,
                                IN[q][:, 1 + CH*n + k: 1 + CH*n + k + CH],
                                start=first, stop=False)
                        first = False
                for j in range(2):
                    lhsT = wt[:, 18 + m*2 + j, :]
                    last = (j == 1 and not residual)
                    for n, pt in enumerate(pts):
                        nc.tensor.matmul(
                            pt, lhsT,
                            IN[2][:, 1 + j + CH*n: 1 + j + CH*n + CH],
                            start=False, stop=last)
                if residual:
                    for n, pt in enumerate(pts):
                        nc.tensor.matmul(
                            pt, ident, OUT[m][:, 2 + CH*n: 2 + CH*(n+1)],
                            start=False, stop=True)
                bias = colap(f'res_b{c4}_{m}')
                for n, pt in enumerate(pts):
                    nc.scalar.activation(
                        out=OUT[m][:, 2 + CH*n: 2 + CH*(n+1)], in_=pt,
                        func=AF.Relu, bias=bias, scale=1.0)
            if c4 < 3:
                dup2(OUT[2])

        h = [None] * BL
        sums = [None] * BL

        def proj(b):
            h[b] = rwt(f'h{b}')
            sums[b] = small.tile([128, 1], F32, tag='hsum', bufs=16,
                                 name=f'hsum{b}')
            for n in range(NCH):
                pt = psum.tile([128, CH], F32, tag='cvp', name=f'pj{b}_{n}')
                for q in range(3):
                    nc.tensor.matmul(pt, w_proj[:, q, :],
                                     F[b][q][:, 2 + CH*n: 2 + CH*(n+1)],
                                     start=(q == 0), stop=(q == 2))
                nc.scalar.activation(out=h[b][:, CH*n:CH*(n+1)], in_=pt,
                                     func=AF.Identity, bias=colap('proj_b'),
                                     scale=1.0)
            nc.vector.tensor_reduce(out=sums[b], in_=h[b][:, 0:T],
                                    axis=mybir.AxisListType.X, op=ALU.add)

        # ---------------- emit: half 1 ------------------------------------
        band(0); band(1)
        pump_q = []

        def pump(n=1):
            for _ in range(n):
                if pump_q:
                    pump_q.pop(0)()

        for c4 in range(4):
            conv(c4, 0, 0)
            conv(c4, 1, 0)
            if c4 + 2 <= 3:
                wres_load((0, c4 + 2))
            else:
                wres_load((1, c4 - 2))
        proj(0); proj(1)
        band(2); band(3)

        def mk_conv(c4, b):
            def f():
                conv(c4, b, 1)
                if b == 3 and c4 + 2 <= 3:
                    wres_load((1, c4 + 2))
            return f

        for c4 in range(4):
            pump_q.append(mk_conv(c4, 2))
            pump_q.append(mk_conv(c4, 3))
        pump_q.append(lambda: proj(2))
        pump_q.append(lambda: proj(3))

        # ---------------- rwkv ---------------------------------------------
        def rwkv_layer(pair, pi, l):
            """Generator: yields at stage boundaries so two pair-streams can
            be interleaved instruction-stream-wise (stall filling)."""
            b0, b1 = pair
            sumsb = small.tile([128, 2], F32R, tag='sumsb', bufs=2,
                               name=f'sumsb{pi}_{l}')
            for j, b in enumerate(pair):
                nc.vector.tensor_copy(out=sumsb[:, j:j+1], in_=sums[b])
            bias_ps = psum.tile([128, 6], F32, tag='rwp', name=f'bps{pi}_{l}')
            for w in range(3):
                nc.tensor.matmul(bias_ps[:, 2*w:2*w+2], w_rwkvb[:, l, w, :],
                                 sumsb)
            bias_sb = small.tile([128, 6], F32, tag='biassb', bufs=2,
                                 name=f'bsb{pi}_{l}')
            nc.scalar.activation(out=bias_sb, in_=bias_ps, func=AF.Copy,
                                 scale=1.0)

            kvr = {}
            for j, b in enumerate(pair):
                kvr[b] = [rwt(f'sk{l}_{b}'), rwt(f'vv{l}_{b}'),
                          rwt(f'rr{l}_{b}')]
            for w, fn in [(0, AF.Sigmoid), (1, AF.Relu), (2, AF.Sigmoid)]:
                for j, b in enumerate(pair):
                    for n in range(NCH):
                        pt = psum.tile([128, CH], F32, tag='rwp',
                                       name=f'kvr{l}_{b}_{w}_{n}')
                        nc.tensor.matmul(pt, w_rwkv[:, l, w, :],
                                         h[b][:, CH*n:CH*(n+1)])
                        nc.scalar.activation(
                            out=kvr[b][w][:, CH*n:CH*(n+1)], in_=pt, func=fn,
                            bias=bias_sb[:, 2*w+j:2*w+j+1], scale=1.0)
            pump()
            yield

            ss = {}; alpha = {}
            for b in pair:
                sk, vv, rr = kvr[b]
                ss[b] = rwt(f'ss{l}_{b}')
                alpha[b] = rwt(f'al{l}_{b}')
                nc.vector.scalar_tensor_tensor(
                    out=ss[b][:, 0:T], in0=sk[:, 0:T], scalar=0.5,
                    in1=vv[:, 0:T], op0=ALU.max, op1=ALU.mult)
                nc.gpsimd.memset(alpha[b][:, 0:1], 0.0)
                nc.vector.tensor_tensor_scan(
                    out=alpha[b][:, 1:T+1], data0=decay, data1=ss[b][:, 0:T],
                    initial=0.0, op0=ALU.mult, op1=ALU.add)
                # wkv into the (dead) sk slot, r*wkv into the (dead) vv slot
                nc.vector.scalar_tensor_tensor(
                    out=sk[:, 0:T], in0=alpha[b][:, 0:T], scalar=0.1,
                    in1=alpha[b][:, 1:T+1], op0=ALU.mult, op1=ALU.add)
                nc.vector.tensor_tensor(out=vv[:, 0:T], in0=rr[:, 0:T],
                                        in1=sk[:, 0:T], op=ALU.mult)
            pump()
            yield

            y = {}; ysq = {}
            for b in pair:
                y[b] = rwt(f'y{l}_{b}')
                prod = kvr[b][1]
                for n in range(NCH):
                    pt = psum.tile([128, CH], F32, tag='rwp',
                                   name=f'yp{l}_{b}_{n}')
                    nc.tensor.matmul(pt, w_rwkv[:, l, 3, :],
                                     prod[:, CH*n:CH*(n+1)], start=True,
                                     stop=False)
                    nc.tensor.matmul(pt, ident, h[b][:, CH*n:CH*(n+1)],
                                     start=False, stop=True)
                    nc.scalar.activation(out=y[b][:, CH*n:CH*(n+1)], in_=pt,
                                         func=AF.Copy, scale=1.0)
                ysq[b] = rwt(f'ysq{l}_{b}')
                nc.scalar.activation(out=ysq[b][:, 0:T], in_=y[b][:, 0:T],
                                     func=AF.Square)
            pump()
            yield

            yn = {b: rwt(f'yn{l}_{b}') for b in pair}
            yield from _ln(nc, rwt, psum, st3, colap, ones_l, ones_lf,
                           pair, pi, y, ysq, yn, f'ln1g_{l}', f'ln1b_{l}',
                           f'l{l}a', pump)
            yield
            ysq2 = {}
            for b in pair:
                ysq2[b] = rwt(f'ysq2{l}_{b}')
                nc.scalar.activation(out=ysq2[b][:, 0:T], in_=yn[b][:, 0:T],
                                     func=AF.Square)
            ffp = {b: rwt(f'ffp{l}_{b}') for b in pair}
            yield from _ln(nc, rwt, psum, st3, colap, ones_l, ones_lf,
                           pair, pi, yn, ysq2, ffp, f'ln2g_{l}', f'ln2b_{l}',
                           f'l{l}b', pump)
            yield

            hn = {b: rwt(f'hn{l}_{b}') for b in pair}
            nsums = {b: small.tile([128, 1], F32, tag='hsum', bufs=16,
                                   name=f'ns{l}_{b}') for b in pair}
            for b in pair:
                nc.vector.scalar_tensor_tensor(
                    out=hn[b][:, 0:T], in0=ffp[b][:, 0:T], scalar=0.0,
                    in1=yn[b][:, 0:T], op0=ALU.max, op1=ALU.add,
                    accum_out=nsums[b])
                h[b] = hn[b]
                sums[b] = nsums[b]

        # pair (0,1) leads pair (2,3) by one layer; stages of the two pairs
        # are round-robin interleaved so each engine's in-order queue
        # alternates short chunks and cross-engine stalls get filled.
        def drive(*gens):
            live = list(gens)
            while live:
                for g in list(live):
                    try:
                        next(g)
                    except StopIteration:
                        live.remove(g)

        drive(rwkv_layer((0, 1), 0, 0))
        pump(len(pump_q))
        drive(rwkv_layer((0, 1), 0, 1), rwkv_layer((2, 3), 1, 0))
        drive(rwkv_layer((0, 1), 0, 2), rwkv_layer((2, 3), 1, 1))
        drive(rwkv_layer((2, 3), 1, 2))

        # ---------------- head ------------------------------------
        pooledHf = small.tile([128, BL], F32R)
        for b in range(BL):
            nc.vector.tensor_scalar(out=pooledHf[:, b:b+1], in0=sums[b],
                                    scalar1=1.0 / T, scalar2=None,
                                    op0=ALU.mult)
        hidT = small.tile([128, 2, BL], F32R)
        for mt in range(2):
            pt = psum.tile([128, BL], F32, tag='rwp', name=f'clsp{mt}')
            nc.tensor.matmul(pt, w_cls1[:, mt, :], pooledHf)
            nc.scalar.activation(out=hidT[:, mt, :], in_=pt, func=AF.Relu,
                                 bias=colap('cls_b1a' if mt == 0 else 'cls_b1b'),
                                 scale=1.0)
        out_ps = psum.tile([NCLS, BL], F32, tag='rwp', name='out_ps')
        for kt in range(2):
            nc.tensor.matmul(out_ps, w_cls2[:, kt, :],
                             hidT[:, kt, :],
                             start=(kt == 0), stop=(kt == 1))
        out_sb = small.tile([NCLS, BL], F32)
        nc.scalar.activation(out=out_sb, in_=out_ps, func=AF.Identity,
                             bias=colap('cls_b2')[0:NCLS], scale=1.0)
        nc.gpsimd.dma_start(out=d_out[:, :], in_=out_sb)


def _ln(nc, rwt, psum, st3, colap, ones_l, ones_lf,
        pair, pi, y, ysq, out, gname, bname, tagp, pump):
    """LayerNorm over the partition axis for each (b, t) column.
    Stats rows live at partition 32*(b%2) + 64*pi of shared (128, T) tiles;
    pair 0 uses rows 0:64, pair 1 rows 64:128 (disjoint -> no cross deps)."""
    sty, stq, stv = st3
    base = 64 * pi
    rows = (base, base + 32)
    sl = slice(base, base + 64)

    for n in range(NCH):
        p1 = psum.tile([128, CH], F32, tag='rwp', name=f'st1_{tagp}_{n}')
        p2 = psum.tile([128, CH], F32, tag='rwp', name=f'st2_{tagp}_{n}')
        for j, b in enumerate(pair):
            r = rows[j]
            nc.tensor.matmul(p1[r:r+1, :], ones_l,
                             y[b][:, CH*n:CH*(n+1)], tile_position=(0, r))
            nc.tensor.matmul(p2[r:r+1, :], ones_l,
                             ysq[b][:, CH*n:CH*(n+1)], tile_position=(0, r))
        c = slice(CH*n, CH*(n+1))
        nc.scalar.activation(out=sty[sl, c], in_=p1[sl, :], func=AF.Copy,
                             scale=1.0 / H)
        nc.scalar.activation(out=stv[sl, c], in_=p1[sl, :], func=AF.Square,
                             scale=1.0 / H)
        nc.vector.scalar_tensor_tensor(
            out=stq[sl, c], in0=p2[sl, :], scalar=1.0 / H, in1=stv[sl, c],
            op0=ALU.mult, op1=ALU.subtract)
    # sigma = sqrt(var+eps); inv = 1/sigma (DVE; ACT Rsqrt is banned and an
    # Ln/Exp route thrashes table sets)
    nc.scalar.activation(out=stv[sl, :], in_=stq[sl, :], func=AF.Sqrt,
                         bias=colap('eps')[sl], scale=1.0)
    with nc.allow_low_precision(reason='fp32r LN inv is plenty (FP22)'):
        nc.vector.reciprocal(out=stq[sl, :], in_=stv[sl, :])
    # negq = -mu * inv  (into stv; sigma there is dead)
    nc.vector.scalar_tensor_tensor(
        out=stv[sl, :], in0=sty[sl, :], scalar=-1.0, in1=stq[sl, :],
        op0=ALU.mult, op1=ALU.mult)
    inv, negq = stq, stv
    gv = colap(gname); bv = colap(bname)
    pump()
    yield
    for j, b in enumerate(pair):
        r = rows[j]
        pb = rwt(f'bcP{tagp}_{b}')
        qb = rwt(f'bcQ{tagp}_{b}')
        for n in range(NCH):
            c = slice(CH*n, CH*(n+1))
            bp = psum.tile([128, CH], F32, tag='rwp', name=f'bp_{tagp}_{b}_{n}')
            bq = psum.tile([128, CH], F32, tag='rwp', name=f'bq_{tagp}_{b}_{n}')
            nc.tensor.matmul(bp, ones_lf[r:r+1, :], inv[r:r+1, c],
                             tile_position=(r, 0))
            nc.tensor.matmul(bq, ones_lf[r:r+1, :], negq[r:r+1, c],
                             tile_position=(r, 0))
            nc.scalar.activation(out=pb[:, c], in_=bp, func=AF.Identity,
                                 bias=0.0, scale=gv)
            nc.vector.tensor_scalar(out=qb[:, c], in0=bq, scalar1=gv,
                                    scalar2=bv, op0=ALU.mult, op1=ALU.add)
        # tmp = y*pb into the dead ysq slot
        nc.vector.tensor_tensor(out=ysq[b][:, 0:T], in0=y[b][:, 0:T],
                                in1=pb[:, 0:T], op=ALU.mult)
        nc.vector.tensor_tensor(out=out[b][:, 0:T], in0=ysq[b][:, 0:T],
                                in1=qb[:, 0:T], op=ALU.add)


# ---------------------------------------------------------------------------
# entry point
# ---------------------------------------------------------------------------

_CACHE = {}


def kernel(**inputs):
    import hashlib
    wkey = hashlib.sha256()
    for k in sorted(inputs):
        if k != 'x':
            wkey.update(np.ascontiguousarray(np.asarray(inputs[k])).tobytes())
    wkey = wkey.hexdigest()
    if _CACHE.get('wkey') != wkey:
        prep = _prep_weights(inputs)
        nv = prep['cvec'].shape[1]
        _CACHE['nc'] = _build_nc(nv, prep)
        _CACHE['wkey'] = wkey
    nc = _CACHE['nc']

    x = np.asarray(inputs['x'], np.float32).astype(bf16np)
    xc = x.reshape(NCORE, BL, T, C).transpose(0, 1, 3, 2)   # (core, b, c, t)
    in_maps = [{'x': np.ascontiguousarray(xc[c])} for c in range(NCORE)]
    _CACHE['in_maps'] = in_maps
    res = run_bass_kernel_spmd(nc, in_maps, core_ids=list(range(NCORE)))
    outs = [res.results[c]['out'] for c in range(NCORE)]   # (NCLS, BL) each
    logits = np.concatenate([o.T for o in outs], axis=0)   # (B, NCLS)
    return logits.astype(np.float32)


def bench_exec(n=8):
    """Steady-state timing of the compiled SPMD executable (device-resident
    inputs, jit built once). Returns (min_s, avg_s) per call."""
    import time
    import jax
    from jax.sharding import Mesh, PartitionSpec
    from jax.experimental.shard_map import shard_map
    from concourse import bass2jax as b2j

    nc = _CACHE['nc']; in_maps = _CACHE['in_maps']
    b2j.install_neuronx_cc_hook()
    partition_name = nc.partition_id_tensor.name if nc.partition_id_tensor else None
    in_names, out_names, out_avals, zero_outs = [], [], [], []
    for alloc in nc.m.functions[0].allocations:
        if not isinstance(alloc, mybir.MemoryLocationSet):
            continue
        name = alloc.memorylocations[0].name
        if alloc.kind == 'ExternalInput':
            if name != partition_name:
                in_names.append(name)
        elif alloc.kind == 'ExternalOutput':
            sh = tuple(alloc.tensor_shape)
            dt = mybir.dt.np(alloc.dtype)
            out_avals.append(jax.core.ShapedArray(sh, dt))
            out_names.append(name)
            zero_outs.append(np.zeros(sh, dt))
    n_params = len(in_names)
    n_outs = len(out_avals)
    all_in_names = list(in_names) + list(out_names)
    if partition_name is not None:
        all_in_names.append(partition_name)

    def _body(*args):
        operands = list(args)
        if partition_name is not None:
            operands.append(b2j.partition_id_tensor())
        outs = b2j._bass_exec_p.bind(
            *operands, out_avals=tuple(out_avals), in_names=tuple(all_in_names),
            out_names=tuple(out_names), lowering_input_output_aliases=(),
            sim_require_finite=True, sim_require_nnan=True, nc=nc)
        return tuple(outs)

    devices = jax.devices()[:NCORE]
    mesh = Mesh(np.asarray(devices), ('core',))
    in_specs = (PartitionSpec('core'),) * (n_params + n_outs)
    out_specs = (PartitionSpec('core'),) * len(out_names)
    sharded = jax.jit(shard_map(_body, mesh=mesh, in_specs=in_specs,
                                out_specs=out_specs, check_rep=False),
                      keep_unused=True)
    concat_in = [np.concatenate([np.asarray(in_maps[c][nm])
                                 for c in range(NCORE)], axis=0)
                 for nm in in_names]
    concat_zeros = [np.zeros((NCORE * z.shape[0], *z.shape[1:]), z.dtype)
                    for z in zero_outs]
    args = [jax.device_put(a) for a in concat_in + concat_zeros]
    r = sharded(*args); jax.block_until_ready(r)   # warmup/compile
    def run_n(k):
        t0 = time.perf_counter()
        rs = [sharded(*args) for _ in range(k)]
        jax.block_until_ready(rs)
        return time.perf_counter() - t0
    run_n(2)
    t1 = min(run_n(1) for _ in range(3))
    tn = min(run_n(n) for _ in range(3))
    slope = (tn - t1) / (n - 1)
    return t1, slope


# revision 37
# speedup vs baseline: 1.1795x; 1.1795x over previous
"""Trainium2 Bass kernel for nn_EEGMI_RWKV_ResNet_Model.

Sharding: data-parallel over batch. B=32 -> 4 batches on each of 8 cores.
All parameters are baked into the NEFF via inline_tensor (loaded to HBM at
model load); only x ships per exec as (BL, 64, T) bf16.

Structure (per core):
  - band conv (depthwise 5-tap) on DVE with attention-prescaled coefficients
  - resnet convs on PE (contraction blocks), residual adds folded into PSUM
    via identity matmuls, relu+bias via ACT on the PSUM->SBUF move
  - rwkv: token-mix folded into row-scaled weight copies (W' = W diag(tm)),
    per-batch mix bias via tiny matmuls; wkv scan on DVE
    (tensor_tensor_scan); LN stats via ones-matmuls + Ln/Exp inverse-stddev;
    per-t broadcast via K=1 matmuls
  - emission is software-pipelined: batches 0,1 flow band->conv->rwkv while
    batches 2,3's convs are pumped into the PE stream between rwkv groups.
"""
import numpy as np
import ml_dtypes

import concourse.bass as bass
import concourse.bacc as bacc
import concourse.tile as tile
from concourse import mybir
from concourse.bass_utils import run_bass_kernel_spmd

EPS = 1e-5
B, T, C = 32, 2048, 64
NB, C5, H, L, NBLK, NCLS = 5, 320, 128, 3, 2, 4
CP = 384
NCORE = 8
BL = B // NCORE
NCH = 4
CH = 512
TP = T + 4      # padded width for band conv input
TF = T + 4      # feat tiles width (data cols 2..2050)

PERM = np.array([(o % 64) * 5 + (o // 64) for o in range(C5)], dtype=np.int64)

F32 = mybir.dt.float32
F32R = mybir.dt.float32r
BF16 = mybir.dt.bfloat16
AF = mybir.ActivationFunctionType
ALU = mybir.AluOpType
bf16np = ml_dtypes.bfloat16


# ---------------------------------------------------------------------------
# host-side weight preprocessing (numpy only)
# ---------------------------------------------------------------------------

def _prep_weights(inp):
    f32 = np.float32
    out = {}

    bw = np.asarray(inp['band_w'], f32)[PERM, 0, :]
    bb = np.asarray(inp['band_b'], f32)[PERM]
    bw_pad = np.zeros((CP, 5), f32); bw_pad[:C5] = bw
    bb_pad = np.zeros((CP,), f32); bb_pad[:C5] = bb
    band_coef = bw_pad.reshape(3, 128, 5)

    bw_raw = np.asarray(inp['band_w'], f32)[:, 0, :].reshape(C, NB, 5)
    denom = f32(1.0 / (NB * T))
    A = bw_raw.sum(axis=(1, 2)) * denom
    E0 = -(bw_raw[:, :, 3] + bw_raw[:, :, 4]).sum(1) * denom
    E1 = -(bw_raw[:, :, 4]).sum(1) * denom
    E2 = -(bw_raw[:, :, 0]).sum(1) * denom
    E3 = -(bw_raw[:, :, 0] + bw_raw[:, :, 1]).sum(1) * denom
    Bb = np.asarray(inp['band_b'], f32).reshape(C, NB).mean(1)

    attn_rhs = np.zeros((65, 64), f32)
    attn_rhs[:64] = np.asarray(inp['attn_w'], f32).T
    attn_rhs[64] = np.asarray(inp['attn_b'], f32)
    out['attn_rhs'] = attn_rhs

    # q2-packed conv weights: per c4, 24 lhsT blocks of (128,128):
    #   idx (k*2+q)*3+mm  for q in {0,1}, k in 0..2  (18 blocks)
    #   idx 18 + mm*2 + j for the q=2 block: j=0 packs [k0-rows; k2-rows]
    #   (rhs rows 64:128 hold a shift-by-2 duplicate of block-2 channels),
    #   j=1 is [k1-rows; zeros].
    res_lhsT = np.zeros((4, 24, 128, 128), f32)
    res_bias = np.zeros((4, CP), f32)
    ci = 0
    for blk in range(NBLK):
        for lyr in range(2):
            W = np.asarray(inp['res_w'], np.float32)[blk, lyr]
            g = np.asarray(inp['res_bn_g'], f32)[blk, lyr]
            b = np.asarray(inp['res_bn_b'], f32)[blk, lyr]
            m = np.asarray(inp['res_bn_m'], f32)[blk, lyr]
            v = np.asarray(inp['res_bn_v'], f32)[blk, lyr]
            inv = g / np.sqrt(v + EPS)
            Wf = W * inv[:, None, None]
            bf = b - m * inv
            Wp = Wf[PERM][:, PERM]
            Wpad = np.zeros((CP, CP, 3), f32); Wpad[:C5, :C5] = Wp
            bpad = np.zeros((CP,), f32); bpad[:C5] = bf[PERM]
            res_bias[ci] = bpad
            WT = Wpad.transpose(1, 0, 2)   # (in_ch, out_ch, k)
            for k in range(3):
                for q in range(2):
                    for mm in range(3):
                        res_lhsT[ci, (k*2+q)*3+mm] = \
                            WT[q*128:(q+1)*128, mm*128:(mm+1)*128, k]
            for mm in range(3):
                mc = slice(mm*128, (mm+1)*128)
                res_lhsT[ci, 18 + mm*2 + 0, 0:64] = WT[256:320, mc, 0]
                res_lhsT[ci, 18 + mm*2 + 0, 64:128] = WT[256:320, mc, 2]
                res_lhsT[ci, 18 + mm*2 + 1, 0:64] = WT[256:320, mc, 1]
            ci += 1
    out['res_lhsT'] = np.ascontiguousarray(
        res_lhsT.transpose(2, 0, 1, 3)).astype(bf16np)

    pw = np.asarray(inp['proj_w'], f32)[:, PERM]
    pw_pad = np.zeros((H, CP), f32); pw_pad[:, :C5] = pw
    out['proj_lhsT'] = np.ascontiguousarray(
        pw_pad.T.reshape(3, 128, H).transpose(1, 0, 2)).astype(bf16np)

    # token-mix fold: W'[w] = W diag(tm_w)  (lhsT rows scaled by tm);
    # bias weights W''[w] = W diag((1-tm_w)/T) for the per-batch mean term.
    rwkv_lhsT = np.zeros((L, 4, H, H), f32)
    rwkv_blhsT = np.zeros((L, 3, H, H), f32)
    tms = [np.asarray(inp['tmk'], f32), np.asarray(inp['tmv'], f32),
           np.asarray(inp['tmr'], f32)]
    ws = [np.asarray(inp['wk'], f32), np.asarray(inp['wv'], f32),
          np.asarray(inp['wr'], f32)]
    for l in range(L):
        for w in range(3):
            rwkv_lhsT[l, w] = ws[w][l].T * tms[w][l][:, None]
            rwkv_blhsT[l, w] = ws[w][l].T * ((1.0 - tms[w][l]) / T)[:, None]
        rwkv_lhsT[l, 3] = np.asarray(inp['wo'], f32)[l].T
    out['rwkv_lhsT'] = np.ascontiguousarray(
        rwkv_lhsT.transpose(2, 0, 1, 3)).astype(bf16np)
    # bias (mean-mix) weights stay f32: sums is a 2048-term reduction and
    # its contribution must not be quantized to bf16
    out['rwkv_blhsT'] = np.ascontiguousarray(
        rwkv_blhsT.transpose(2, 0, 1, 3))

    w1 = np.asarray(inp['cls_w1'], f32)
    out['cls1_lhsT'] = np.ascontiguousarray(w1.T.reshape(H, 2, 128))
    w2 = np.asarray(inp['cls_w2'], f32)
    out['cls2_lhsT'] = np.ascontiguousarray(
        w2.T.reshape(2, 128, NCLS).transpose(1, 0, 2))

    out['ident_bf'] = np.eye(128, dtype=f32).astype(bf16np)

    cols = {}
    def vec(name, v):
        cols[name] = np.asarray(v, f32)
    def pad128(v):
        o = np.zeros(128, f32); o[:len(v)] = v; return o

    for i in range(3):
        for k in range(5):
            vec(f'band_c{i}_{k}', band_coef[i, :, k])
    for i in range(3):
        vec(f'band_bias_{i}', bb_pad.reshape(3, 128)[i])
    vec('A', pad128(A)); vec('E0', pad128(E0)); vec('E1', pad128(E1))
    vec('E2', pad128(E2)); vec('E3', pad128(E3)); vec('Bb', pad128(Bb))
    for c4 in range(4):
        for mm in range(3):
            vec(f'res_b{c4}_{mm}', res_bias[c4, mm*128:(mm+1)*128])
    vec('proj_b', np.asarray(inp['proj_b'], f32))
    for l in range(L):
        vec(f'ln1g_{l}', np.asarray(inp['ln1g'], f32)[l])
        vec(f'ln1b_{l}', np.asarray(inp['ln1b'], f32)[l])
        vec(f'ln2g_{l}', np.asarray(inp['ln2g'], f32)[l])
        vec(f'ln2b_{l}', np.asarray(inp['ln2b'], f32)[l])
    vec('cls_b1a', np.asarray(inp['cls_b1'], f32)[:128])
    vec('cls_b1b', np.asarray(inp['cls_b1'], f32)[128:])
    vec('cls_b2', pad128(np.asarray(inp['cls_b2'], f32)))
    vec('eps', np.full(128, EPS, f32))

    names = list(cols.keys())
    out['cvec'] = np.ascontiguousarray(np.stack([cols[n] for n in names], 1))
    out['cvec_idx'] = {n: i for i, n in enumerate(names)}
    return out


# ---------------------------------------------------------------------------
# bass kernel builder
# ---------------------------------------------------------------------------

def _inline(nc, arr, name, dtype=None):
    h = nc.inline_tensor(np.ascontiguousarray(arr), name=name)
    if dtype is not None and dtype != h.dtype:
        mls = nc.lookup_mls(h)
        mls.dtype = dtype
        h = bass.DRamTensorHandle(h.name, list(arr.shape), dtype)
    return h


def _build_nc(nv, prep):
    nc = bacc.Bacc(None, target_bir_lowering=False)

    d_x = nc.dram_tensor('x', [BL, 64, T], BF16, kind='ExternalInput')
    d = {
        'cvec': _inline(nc, prep['cvec'], 'cvec'),
        'attn': _inline(nc, prep['attn_rhs'], 'attn_rhs', F32R),
        'res': _inline(nc, prep['res_lhsT'], 'res_lhsT'),
        'proj': _inline(nc, prep['proj_lhsT'], 'proj_lhsT'),
        'rwkv': _inline(nc, prep['rwkv_lhsT'], 'rwkv_lhsT'),
        'rwkvb': _inline(nc, prep['rwkv_blhsT'], 'rwkv_blhsT', F32R),
        'cls1': _inline(nc, prep['cls1_lhsT'], 'cls1_lhsT', F32R),
        'cls2': _inline(nc, prep['cls2_lhsT'], 'cls2_lhsT', F32R),
        'ident': _inline(nc, prep['ident_bf'], 'ident_bf'),
    }
    d_out = nc.dram_tensor('out', [NCLS, BL], F32, kind='ExternalOutput')

    with tile.TileContext(nc) as tc:
        _emit(nc, tc, d_x, d, d_out, nv, prep['cvec_idx'])
    nc.finalize()
    return nc


def _emit(nc, tc, d_x, d, d_out, nv, cvi):
    from contextlib import ExitStack

    ctx = ExitStack()
    with ctx:
        consts = ctx.enter_context(tc.tile_pool(name='consts', bufs=1))
        wres = ctx.enter_context(tc.tile_pool(name='wres', bufs=2))
        convp = ctx.enter_context(tc.tile_pool(name='convp', bufs=4))
        fo = ctx.enter_context(tc.tile_pool(name='fo', bufs=12))
        rw = ctx.enter_context(tc.tile_pool(name='rw', bufs=20))
        stats = ctx.enter_context(tc.tile_pool(name='stats', bufs=1))
        small = ctx.enter_context(tc.tile_pool(name='small', bufs=1))
        psum = ctx.enter_context(tc.tile_pool(name='psum', bufs=4, space='PSUM'))

        def fot(name):
            return fo.tile([128, TF], BF16, tag='fo', name=name)

        def rwt(name):
            return rw.tile([128, TF], BF16, tag='rw', name=name)

        # ---------------- constants -----------------
        cvec = consts.tile([128, nv], F32)
        nc.gpsimd.dma_start(out=cvec, in_=d['cvec'][:, :])

        def colap(name):
            i = cvi[name]
            return cvec[:, i:i+1]

        ones_l = consts.tile([128, 1], BF16)
        nc.vector.memset(ones_l, 1.0)
        decay_sm = consts.tile([128, 128], F32)
        nc.vector.memset(decay_sm, 0.9)
        decay = decay_sm[:, 0:1].broadcast_to([128, T])
        # f32r tiles cannot be memset directly; synthesize via ACT Copy
        ones_lf = consts.tile([128, 128], F32R)
        nc.scalar.activation(out=ones_lf, in_=decay_sm, func=AF.Copy,
                             bias=1.0, scale=0.0)

        attn_rhs = consts.tile([65, 64], F32R)
        nc.gpsimd.dma_start(out=attn_rhs, in_=d['attn'][:, :])
        w_proj = consts.tile([128, 3, H], BF16)
        nc.gpsimd.dma_start(out=w_proj, in_=d['proj'][...])
        w_rwkv = consts.tile([128, L, 4, H], BF16)
        nc.gpsimd.dma_start(out=w_rwkv, in_=d['rwkv'][...])
        w_rwkvb = consts.tile([128, L, 3, H], F32R)
        nc.gpsimd.dma_start(out=w_rwkvb, in_=d['rwkvb'][...])
        w_cls1 = consts.tile([128, 2, 128], F32R)
        nc.gpsimd.dma_start(out=w_cls1, in_=d['cls1'][...])
        w_cls2 = consts.tile([128, 2, NCLS], F32R)
        nc.gpsimd.dma_start(out=w_cls2, in_=d['cls2'][...])
        ident = consts.tile([128, 128], BF16)
        nc.gpsimd.dma_start(out=ident, in_=d['ident'][...])

        # shared LN stat tiles: pair 0 uses partitions 0:64, pair 1 64:128;
        # allocated once so row-disjoint ops never falsely serialize.
        sty = stats.tile([128, T], F32, tag='sty', name='sty')
        stq = stats.tile([128, T], F32R, tag='stq', name='stq')
        stv = stats.tile([128, T], F32R, tag='stv', name='stv')
        st3 = (sty, stq, stv)

        # streamed resnet conv weights: ring of 2, 8 loads (2 halves x 4)
        wres_tiles = {}

        def wres_load(tag):
            half, c4 = tag
            t = wres.tile([128, 24, 128], BF16, tag='wres',
                          name=f'wres{half}_{c4}')
            nc.sync.dma_start(out=t, in_=d['res'][:, c4, :, :])
            wres_tiles[tag] = t

        # ---------------- stage 1: load x ------------
        xdup = [convp.tile([128, TP], BF16, tag='xdup', name=f'xdup{b}')
                for b in range(BL)]
        S_b = small.tile([64, BL], F32)
        for b in range(BL):
            nc.gpsimd.memset(xdup[b][:, 0:2], 0.0)
            nc.gpsimd.memset(xdup[b][:, 2+T:4+T], 0.0)
            nc.sync.dma_start(out=xdup[b][0:64, 2:2+T], in_=d_x[b, :, :])
            nc.sync.dma_start(out=xdup[b][64:128, 2:2+T], in_=d_x[b, :, :])
            nc.vector.tensor_reduce(
                out=S_b[:, b:b+1], in_=xdup[b][0:64, 2:2+T],
                axis=mybir.AxisListType.X, op=ALU.add)
        wres_load((0, 0))
        wres_load((0, 1))

        # ---------------- attention ------------------------------------
        pooledT = small.tile([65, BL], F32R)
        nc.scalar.activation(out=pooledT[64:65, :], in_=S_b[0:1, 0:BL],
                             func=AF.Copy, bias=1.0, scale=0.0)
        for b in range(BL):
            p = pooledT[0:64, b:b+1]
            nc.vector.tensor_scalar(
                out=p, in0=S_b[:, b:b+1], scalar1=colap('A')[0:64],
                scalar2=colap('Bb')[0:64], op0=ALU.mult, op1=ALU.add)
            for name, cc in [('E0', 2), ('E1', 3), ('E2', T), ('E3', T+1)]:
                nc.vector.scalar_tensor_tensor(
                    out=p, in0=xdup[b][0:64, cc:cc+1],
                    scalar=colap(name)[0:64], in1=p,
                    op0=ALU.mult, op1=ALU.add)
        att_ps = psum.tile([64, BL], F32, tag='rwp', name='att_ps')
        nc.tensor.matmul(att_ps, attn_rhs, pooledT)
        attE = small.tile([64, BL], F32R)
        nc.scalar.activation(out=attE, in_=att_ps, func=AF.Exp)
        sum_ps = psum.tile([1, BL], F32, tag='rwp', name='sum_ps')
        nc.tensor.matmul(sum_ps, ones_lf[0:64, 0:1], attE)
        arec = small.tile([1, BL], F32R)
        with nc.allow_low_precision(reason='softmax denom in fp32r is fine'):
            nc.vector.reciprocal(out=arec, in_=sum_ps)
        bc_ps = psum.tile([64, BL], F32, tag='rwp', name='bc_ps')
        nc.tensor.matmul(bc_ps, ones_lf[0:1, 0:64], arec, tile_position=(0, 0))
        attT = small.tile([64, BL], F32)
        nc.vector.tensor_tensor(out=attT, in0=attE, in1=bc_ps, op=ALU.mult)
        # attention-scaled band coefficients + biases, per batch
        avec = [small.tile([128, 1], F32, tag='avec', bufs=4, name=f'avec{b}')
                for b in range(BL)]
        cfb = [small.tile([128, 15], F32, tag='cfb', bufs=4, name=f'cfb{b}')
               for b in range(BL)]
        bxa = [small.tile([128, 3], F32, tag='bxa', bufs=4, name=f'bxa{b}')
               for b in range(BL)]
        c0 = cvi['band_c0_0']
        bb0 = cvi['band_bias_0']
        for b in range(BL):
            nc.gpsimd.dma_start(out=avec[b][0:64, :], in_=attT[:, b:b+1])
            nc.gpsimd.dma_start(out=avec[b][64:128, :], in_=attT[:, b:b+1])
            nc.vector.tensor_scalar(
                out=cfb[b], in0=cvec[:, c0:c0+15], scalar1=avec[b],
                scalar2=None, op0=ALU.mult)
            nc.vector.tensor_scalar(
                out=bxa[b], in0=cvec[:, bb0:bb0+3], scalar1=avec[b],
                scalar2=None, op0=ALU.mult)

        # ---------------- band conv (DVE) -------------------------------
        F = [None] * BL

        def dup2(tile_):
            # rows 64:128 <- shift-by-2 duplicate of rows 0:64 (q2 packing)
            nc.sync.dma_start(out=tile_[64:128, 0:TF-2], in_=tile_[0:64, 2:TF])

        def band(b):
            F[b] = [fot(f'F{b}_{i}') for i in range(3)]
            for i in range(3):
                dst = F[b][i][:, 2:2+T]
                nc.gpsimd.memset(F[b][i][:, 0:2], 0.0)
                nc.gpsimd.memset(F[b][i][:, 2+T:4+T], 0.0)
                nc.vector.tensor_scalar(
                    out=dst, in0=xdup[b][:, 0:T],
                    scalar1=cfb[b][:, 5*i:5*i+1],
                    scalar2=bxa[b][:, i:i+1], op0=ALU.mult, op1=ALU.add)
                for k in range(1, 5):
                    nc.vector.scalar_tensor_tensor(
                        out=dst, in0=xdup[b][:, k:k+T],
                        scalar=cfb[b][:, 5*i+k:5*i+k+1], in1=dst,
                        op0=ALU.mult, op1=ALU.add)
            dup2(F[b][2])

        # ---------------- resnet conv stage ------------------------------
        O = [None] * BL

        def conv(c4, b, half):
            wt = wres_tiles[(half, c4)]
            if c4 == 0:
                O[b] = [fot(f'O{b}_{m}') for m in range(3)]
                for m in range(3):
                    nc.gpsimd.memset(O[b][m][:, 0:2], 0.0)
                    nc.gpsimd.memset(O[b][m][:, 2+T:4+T], 0.0)
            IN = F[b] if c4 in (0, 2) else O[b]
            OUT = O[b] if c4 in (0, 2) else F[b]
            residual = c4 in (1, 3)
            for m in range(3):
                pts = [psum.tile([128, CH], F32, tag='cvp',
                                 name=f'cv{c4}_{b}_{m}_{n}')
                       for n in range(NCH)]
                first = True
                for k in range(3):
                    for q in range(2):
                        lhsT = wt[:, (k*2+q)*3 + m, :]
                        for n, pt in enumerate(pts):
                            nc.tensor.matmul(
                                pt, lhsT,
                                IN[q][:, 1 + CH*n + k: 1 + CH*n + k + CH],
                                start=first, stop=False)
                        first = False
                for j in range(2):
                    lhsT = wt[:, 18 + m*2 + j, :]
                    last = (j == 1 and not residual)
                    for n, pt in enumerate(pts):
                        nc.tensor.matmul(
                            pt, lhsT,
                            IN[2][:, 1 + j + CH*n: 1 + j + CH*n + CH],
                            start=False, stop=last)
                if residual:
                    for n, pt in enumerate(pts):
                        nc.tensor.matmul(
                            pt, ident, OUT[m][:, 2 + CH*n: 2 + CH*(n+1)],
                            start=False, stop=True)
                bias = colap(f'res_b{c4}_{m}')
                for n, pt in enumerate(pts):
                    nc.scalar.activation(
                        out=OUT[m][:, 2 + CH*n: 2 + CH*(n+1)], in_=pt,
                        func=AF.Relu, bias=bias, scale=1.0)
            if c4 < 3:
                dup2(OUT[2])

        h = [None] * BL
        sums = [None] * BL

        def proj(b):
            h[b] = rwt(f'h{b}')
            sums[b] = small.tile([128, 1], F32, tag='hsum', bufs=16,
                                 name=f'hsum{b}')
            for n in range(NCH):
                pt = psum.tile([128, CH], F32, tag='cvp', name=f'pj{b}_{n}')
                for q in range(3):
                    nc.tensor.matmul(pt, w_proj[:, q, :],
                                     F[b][q][:, 2 + CH*n: 2 + CH*(n+1)],
                                     start=(q == 0), stop=(q == 2))
                nc.scalar.activation(out=h[b][:, CH*n:CH*(n+1)], in_=pt,
                                     func=AF.Identity, bias=colap('proj_b'),
                                     scale=1.0)
            nc.vector.tensor_reduce(out=sums[b], in_=h[b][:, 0:T],
                                    axis=mybir.AxisListType.X, op=ALU.add)

        # ---------------- emit: half 1 ------------------------------------
        band(0); band(1)
        pump_q = []

        def pump(n=1):
            for _ in range(n):
                if pump_q:
                    pump_q.pop(0)()

        for c4 in range(4):
            conv(c4, 0, 0)
            conv(c4, 1, 0)
            if c4 + 2 <= 3:
                wres_load((0, c4 + 2))
            else:
                wres_load((1, c4 - 2))
        proj(0); proj(1)
        band(2); band(3)

        def mk_conv(c4, b):
            def f():
                conv(c4, b, 1)
                if b == 3 and c4 + 2 <= 3:
                    wres_load((1, c4 + 2))
            return f

        for c4 in range(4):
            pump_q.append(mk_conv(c4, 2))
            pump_q.append(mk_conv(c4, 3))
        pump_q.append(lambda: proj(2))
        pump_q.append(lambda: proj(3))

        # ---------------- rwkv ---------------------------------------------
        def rwkv_layer(pair, pi, l):
            """Generator: yields at stage boundaries so two pair-streams can
            be interleaved instruction-stream-wise (stall filling)."""
            b0, b1 = pair
            sumsb = small.tile([128, 2], F32R, tag='sumsb', bufs=2,
                               name=f'sumsb{pi}_{l}')
            for j, b in enumerate(pair):
                nc.vector.tensor_copy(out=sumsb[:, j:j+1], in_=sums[b])
            bias_ps = psum.tile([128, 6], F32, tag='rwp', name=f'bps{pi}_{l}')
            for w in range(3):
                nc.tensor.matmul(bias_ps[:, 2*w:2*w+2], w_rwkvb[:, l, w, :],
                                 sumsb)
            bias_sb = small.tile([128, 6], F32, tag='biassb', bufs=2,
                                 name=f'bsb{pi}_{l}')
            nc.scalar.activation(out=bias_sb, in_=bias_ps, func=AF.Copy,
                                 scale=1.0)

            kvr = {}
            for j, b in enumerate(pair):
                kvr[b] = [rwt(f'sk{l}_{b}'), rwt(f'vv{l}_{b}'),
                          rwt(f'rr{l}_{b}')]
            for w, fn in [(0, AF.Sigmoid), (1, AF.Relu), (2, AF.Sigmoid)]:
                for j, b in enumerate(pair):
                    for n in range(NCH):
                        pt = psum.tile([128, CH], F32, tag='rwp',
                                       name=f'kvr{l}_{b}_{w}_{n}')
                        nc.tensor.matmul(pt, w_rwkv[:, l, w, :],
                                         h[b][:, CH*n:CH*(n+1)])
                        nc.scalar.activation(
                            out=kvr[b][w][:, CH*n:CH*(n+1)], in_=pt, func=fn,
                            bias=bias_sb[:, 2*w+j:2*w+j+1], scale=1.0)
            pump()
            yield

            ss = {}; alpha = {}
            for b in pair:
                sk, vv, rr = kvr[b]
                ss[b] = rwt(f'ss{l}_{b}')
                alpha[b] = rwt(f'al{l}_{b}')
                nc.vector.scalar_tensor_tensor(
                    out=ss[b][:, 0:T], in0=sk[:, 0:T], scalar=0.5,
                    in1=vv[:, 0:T], op0=ALU.max, op1=ALU.mult)
                nc.gpsimd.memset(alpha[b][:, 0:1], 0.0)
                nc.vector.tensor_tensor_scan(
                    out=alpha[b][:, 1:T+1], data0=decay, data1=ss[b][:, 0:T],
                    initial=0.0, op0=ALU.mult, op1=ALU.add)
                # wkv into the (dead) sk slot, r*wkv into the (dead) vv slot
                nc.vector.scalar_tensor_tensor(
                    out=sk[:, 0:T], in0=alpha[b][:, 0:T], scalar=0.1,
                    in1=alpha[b][:, 1:T+1], op0=ALU.mult, op1=ALU.add)
                nc.vector.tensor_tensor(out=vv[:, 0:T], in0=rr[:, 0:T],
                                        in1=sk[:, 0:T], op=ALU.mult)
            pump()
            yield

            y = {}; ysq = {}
            for b in pair:
                y[b] = rwt(f'y{l}_{b}')
                prod = kvr[b][1]
                for n in range(NCH):
                    pt = psum.tile([128, CH], F32, tag='rwp',
                                   name=f'yp{l}_{b}_{n}')
                    nc.tensor.matmul(pt, w_rwkv[:, l, 3, :],
                                     prod[:, CH*n:CH*(n+1)], start=True,
                                     stop=False)
                    nc.tensor.matmul(pt, ident, h[b][:, CH*n:CH*(n+1)],
                                     start=False, stop=True)
                    nc.scalar.activation(out=y[b][:, CH*n:CH*(n+1)], in_=pt,
                                         func=AF.Copy, scale=1.0)
                ysq[b] = rwt(f'ysq{l}_{b}')
                nc.scalar.activation(out=ysq[b][:, 0:T], in_=y[b][:, 0:T],
                                     func=AF.Square)
            pump()
            yield

            yn = {b: rwt(f'yn{l}_{b}') for b in pair}
            yield from _ln(nc, rwt, psum, st3, colap, ones_l, ones_lf,
                           pair, pi, y, ysq, yn, f'ln1g_{l}', f'ln1b_{l}',
                           f'l{l}a', pump)
            yield
            ysq2 = {}
            for b in pair:
                ysq2[b] = rwt(f'ysq2{l}_{b}')
                nc.scalar.activation(out=ysq2[b][:, 0:T], in_=yn[b][:, 0:T],
                                     func=AF.Square)
            ffp = {b: rwt(f'ffp{l}_{b}') for b in pair}
            yield from _ln(nc, rwt, psum, st3, colap, ones_l, ones_lf,
                           pair, pi, yn, ysq2, ffp, f'ln2g_{l}', f'ln2b_{l}',
                           f'l{l}b', pump)
            yield

            hn = {b: rwt(f'hn{l}_{b}') for b in pair}
            nsums = {b: small.tile([128, 1], F32, tag='hsum', bufs=16,
                                   name=f'ns{l}_{b}') for b in pair}
            for b in pair:
                nc.vector.scalar_tensor_tensor(
                    out=hn[b][:, 0:T], in0=ffp[b][:, 0:T], scalar=0.0,
                    in1=yn[b][:, 0:T], op0=ALU.max, op1=ALU.add,
                    accum_out=nsums[b])
                h[b] = hn[b]
                sums[b] = nsums[b]

        # pair (0,1) leads pair (2,3) by one layer; stages of the two pairs
        # are round-robin interleaved so each engine's in-order queue
        # alternates short chunks and cross-engine stalls get filled.
        def drive(*gens):
            live = list(gens)
            while live:
                for g in list(live):
                    try:
                        next(g)
                    except StopIteration:
                        live.remove(g)

        def chain(*gens):
            for g in gens:
                yield from g

        drive(rwkv_layer((0, 1), 0, 0))
        pump(len(pump_q))
        drive(chain(rwkv_layer((0, 1), 0, 1), rwkv_layer((0, 1), 0, 2)),
              chain(rwkv_layer((2, 3), 1, 0), rwkv_layer((2, 3), 1, 1),
                    rwkv_layer((2, 3), 1, 2)))

        # ---------------- head ------------------------------------
        pooledHf = small.tile([128, BL], F32R)
        for b in range(BL):
            nc.vector.tensor_scalar(out=pooledHf[:, b:b+1], in0=sums[b],
                                    scalar1=1.0 / T, scalar2=None,
                                    op0=ALU.mult)
        hidT = small.tile([128, 2, BL], F32R)
        for mt in range(2):
            pt = psum.tile([128, BL], F32, tag='rwp', name=f'clsp{mt}')
            nc.tensor.matmul(pt, w_cls1[:, mt, :], pooledHf)
            nc.scalar.activation(out=hidT[:, mt, :], in_=pt, func=AF.Relu,
                                 bias=colap('cls_b1a' if mt == 0 else 'cls_b1b'),
                                 scale=1.0)
        out_ps = psum.tile([NCLS, BL], F32, tag='rwp', name='out_ps')
        for kt in range(2):
            nc.tensor.matmul(out_ps, w_cls2[:, kt, :],
                             hidT[:, kt, :],
                             start=(kt == 0), stop=(kt == 1))
        out_sb = small.tile([NCLS, BL], F32)
        nc.scalar.activation(out=out_sb, in_=out_ps, func=AF.Identity,
                             bias=colap('cls_b2')[0:NCLS], scale=1.0)
        nc.gpsimd.dma_start(out=d_out[:, :], in_=out_sb)


def _ln(nc, rwt, psum, st3, colap, ones_l, ones_lf,
        pair, pi, y, ysq, out, gname, bname, tagp, pump):
    """LayerNorm over the partition axis for each (b, t) column.
    Stats rows live at partition 32*(b%2) + 64*pi of shared (128, T) tiles;
    pair 0 uses rows 0:64, pair 1 rows 64:128 (disjoint -> no cross deps)."""
    sty, stq, stv = st3
    base = 64 * pi
    rows = (base, base + 32)
    sl = slice(base, base + 64)

    for n in range(NCH):
        p1 = psum.tile([128, CH], F32, tag='rwp', name=f'st1_{tagp}_{n}')
        p2 = psum.tile([128, CH], F32, tag='rwp', name=f'st2_{tagp}_{n}')
        for j, b in enumerate(pair):
            r = rows[j]
            nc.tensor.matmul(p1[r:r+1, :], ones_l,
                             y[b][:, CH*n:CH*(n+1)], tile_position=(0, r))
            nc.tensor.matmul(p2[r:r+1, :], ones_l,
                             ysq[b][:, CH*n:CH*(n+1)], tile_position=(0, r))
        c = slice(CH*n, CH*(n+1))
        nc.scalar.activation(out=sty[sl, c], in_=p1[sl, :], func=AF.Copy,
                             scale=1.0 / H)
        nc.scalar.activation(out=stv[sl, c], in_=p1[sl, :], func=AF.Square,
                             scale=1.0 / H)
        nc.vector.scalar_tensor_tensor(
            out=stq[sl, c], in0=p2[sl, :], scalar=1.0 / H, in1=stv[sl, c],
            op0=ALU.mult, op1=ALU.subtract)
    # sigma = sqrt(var+eps); inv = 1/sigma (DVE; ACT Rsqrt is banned and an
    # Ln/Exp route thrashes table sets)
    nc.scalar.activation(out=stv[sl, :], in_=stq[sl, :], func=AF.Sqrt,
                         bias=colap('eps')[sl], scale=1.0)
    with nc.allow_low_precision(reason='fp32r LN inv is plenty (FP22)'):
        nc.vector.reciprocal(out=stq[sl, :], in_=stv[sl, :])
    # negq = -mu * inv  (into stv; sigma there is dead)
    nc.vector.scalar_tensor_tensor(
        out=stv[sl, :], in0=sty[sl, :], scalar=-1.0, in1=stq[sl, :],
        op0=ALU.mult, op1=ALU.mult)
    inv, negq = stq, stv
    gv = colap(gname); bv = colap(bname)
    pump()
    yield
    for j, b in enumerate(pair):
        r = rows[j]
        pb = rwt(f'bcP{tagp}_{b}')
        qb = rwt(f'bcQ{tagp}_{b}')
        for n in range(NCH):
            c = slice(CH*n, CH*(n+1))
            bp = psum.tile([128, CH], F32, tag='rwp', name=f'bp_{tagp}_{b}_{n}')
            bq = psum.tile([128, CH], F32, tag='rwp', name=f'bq_{tagp}_{b}_{n}')
            nc.tensor.matmul(bp, ones_lf[r:r+1, :], inv[r:r+1, c],
                             tile_position=(r, 0))
            nc.tensor.matmul(bq, ones_lf[r:r+1, :], negq[r:r+1, c],
                             tile_position=(r, 0))
            nc.scalar.activation(out=pb[:, c], in_=bp, func=AF.Identity,
                                 bias=0.0, scale=gv)
            nc.vector.tensor_scalar(out=qb[:, c], in0=bq, scalar1=gv,
                                    scalar2=bv, op0=ALU.mult, op1=ALU.add)
        # tmp = y*pb into the dead ysq slot
        nc.vector.tensor_tensor(out=ysq[b][:, 0:T], in0=y[b][:, 0:T],
                                in1=pb[:, 0:T], op=ALU.mult)
        nc.vector.tensor_tensor(out=out[b][:, 0:T], in0=ysq[b][:, 0:T],
                                in1=qb[:, 0:T], op=ALU.add)


# ---------------------------------------------------------------------------
# entry point
# ---------------------------------------------------------------------------

_CACHE = {}


def kernel(**inputs):
    import hashlib
    wkey = hashlib.sha256()
    for k in sorted(inputs):
        if k != 'x':
            wkey.update(np.ascontiguousarray(np.asarray(inputs[k])).tobytes())
    wkey = wkey.hexdigest()
    if _CACHE.get('wkey') != wkey:
        prep = _prep_weights(inputs)
        nv = prep['cvec'].shape[1]
        _CACHE['nc'] = _build_nc(nv, prep)
        _CACHE['wkey'] = wkey
    nc = _CACHE['nc']

    x = np.asarray(inputs['x'], np.float32).astype(bf16np)
    xc = x.reshape(NCORE, BL, T, C).transpose(0, 1, 3, 2)   # (core, b, c, t)
    in_maps = [{'x': np.ascontiguousarray(xc[c])} for c in range(NCORE)]
    _CACHE['in_maps'] = in_maps
    res = run_bass_kernel_spmd(nc, in_maps, core_ids=list(range(NCORE)))
    outs = [res.results[c]['out'] for c in range(NCORE)]   # (NCLS, BL) each
    logits = np.concatenate([o.T for o in outs], axis=0)   # (B, NCLS)
    return logits.astype(np.float32)


def bench_exec(n=8):
    """Steady-state timing of the compiled SPMD executable (device-resident
    inputs, jit built once). Returns (min_s, avg_s) per call."""
    import time
    import jax
    from jax.sharding import Mesh, PartitionSpec
    from jax.experimental.shard_map import shard_map
    from concourse import bass2jax as b2j

    nc = _CACHE['nc']; in_maps = _CACHE['in_maps']
    b2j.install_neuronx_cc_hook()
    partition_name = nc.partition_id_tensor.name if nc.partition_id_tensor else None
    in_names, out_names, out_avals, zero_outs = [], [], [], []
    for alloc in nc.m.functions[0].allocations:
        if not isinstance(alloc, mybir.MemoryLocationSet):
            continue
        name = alloc.memorylocations[0].name
        if alloc.kind == 'ExternalInput':
            if name != partition_name:
                in_names.append(name)
        elif alloc.kind == 'ExternalOutput':
            sh = tuple(alloc.tensor_shape)
            dt = mybir.dt.np(alloc.dtype)
            out_avals.append(jax.core.ShapedArray(sh, dt))
            out_names.append(name)
            zero_outs.append(np.zeros(sh, dt))
    n_params = len(in_names)
    n_outs = len(out_avals)
    all_in_names = list(in_names) + list(out_names)
    if partition_name is not None:
        all_in_names.append(partition_name)

    def _body(*args):
        operands = list(args)
        if partition_name is not None:
            operands.append(b2j.partition_id_tensor())
        outs = b2j._bass_exec_p.bind(
            *operands, out_avals=tuple(out_avals), in_names=tuple(all_in_names),
            out_names=tuple(out_names), lowering_input_output_aliases=(),
            sim_require_finite=True, sim_require_nnan=True, nc=nc)
        return tuple(outs)

    devices = jax.devices()[:NCORE]
    mesh = Mesh(np.asarray(devices), ('core',))
    in_specs = (PartitionSpec('core'),) * (n_params + n_outs)
    out_specs = (PartitionSpec('core'),) * len(out_names)
    sharded = jax.jit(shard_map(_body, mesh=mesh, in_specs=in_specs,
                                out_specs=out_specs, check_rep=False),
                      keep_unused=True)
    concat_in = [np.concatenate([np.asarray(in_maps[c][nm])
                                 for c in range(NCORE)], axis=0)
                 for nm in in_names]
    concat_zeros = [np.zeros((NCORE * z.shape[0], *z.shape[1:]), z.dtype)
                    for z in zero_outs]
    args = [jax.device_put(a) for a in concat_in + concat_zeros]
    r = sharded(*args); jax.block_until_ready(r)   # warmup/compile
    def run_n(k):
        t0 = time.perf_counter()
        rs = [sharded(*args) for _ in range(k)]
        jax.block_until_ready(rs)
        return time.perf_counter() - t0
    run_n(2)
    t1 = min(run_n(1) for _ in range(3))
    tn = min(run_n(n) for _ in range(3))
    slope = (tn - t1) / (n - 1)
    return t1, slope
